# revision 1
# baseline (speedup 1.0000x reference)
"""MoE (E=4 experts, top-2 routing) forward pass on 8 Trainium2 NeuronCores.

Strategy: data-parallel over tokens. Full input x is [8, 2048, 1024]; core i
processes batch row i (2048 tokens). Expert weights are replicated to every
core. All experts are computed densely per token (E=4, top-2 -> 2x extra
matmul work, but no data-dependent routing), then combined with the top-2
softmax weights.

Per-core pipeline (T=2048 tokens, D=1024, E=4):
  prologue: PE-transpose x into x^T (bf16 for matmul lhsT + fp32 for gating),
            fp32 gate matmuls -> top-2 softmax weights (fp32: routing needs
            fp32 precision; min top2/top3 score gap on real data is ~2e-5).
  per (expert, token-tile):
    z    = x @ W1 + b1        PE, bf16 operands, fp32 PSUM (b1 via K=1 matmul)
    LN1 stats                 DVE bn_stats/bn_aggr reading PSUM
    n1   = (z - m)*rstd       ACT (per-partition scale/bias), PSUM -> SBUF
    n1  *= g1                 DVE tensor_tensor (in-place)
    n1  += be1                GPSIMD tensor_tensor (in-place)
    u    = relu(n1)           ACT, bf16 out
    u^T                       PE transpose (8x 128x128 bf16 blocks)
    z2   = u @ W2 + b2        PE bf16
    LN2 stats                 DVE
    n2   = (z2 - m2)*rstd2*w_e  ACT (w_e folded into the scale)
    n2  *= g2                 DVE (in-place)
    acc += n2                 GPSIMD (e=0 does acc = n2 + x residual)
  finalize per token-tile:
    C    = w @ be2            PE (K=4 matmul over experts)
    out  = C + acc            DVE, then DMA out
"""

import threading

import numpy as np

import concourse.bass as bass
import concourse.mybir as mybir
import concourse.tile as tile
from concourse import bacc
from concourse.bass import ds, ts
from concourse.masks import make_identity

F32 = mybir.dt.float32
BF16 = mybir.dt.bfloat16
AF = mybir.ActivationFunctionType
ALU = mybir.AluOpType
AX = mybir.AxisListType

P = 128
D = 1024
E = 4
KC = D // P  # contraction chunks per matmul
NCH = D // 512  # psum column chunks
LN_EPS = 1e-5
N_CORES = 8


def _row1(ap):
    """Lift an AP to have a leading length-1 (partition) dim."""
    return bass.AP(tensor=ap.tensor, offset=ap.offset, ap=[[0, 1]] + list(ap.ap))


def _bcast_rows(ap_row, p=P):
    """Broadcast a [1, N]-ish DRAM AP across p partitions (step-0 partition dim)."""
    inner = [list(d) for d in ap_row.ap if d[1] != 1]
    return bass.AP(tensor=ap_row.tensor, offset=ap_row.offset, ap=[[0, p]] + inner)


def build_moe_nc(T=2048, num_devices=N_CORES):
    TT = T // P
    nc = bacc.Bacc(
        "TRN2", target_bir_lowering=False, debug=False, num_devices=num_devices
    )

    x_d = nc.dram_tensor("x", [T, D], F32, kind="ExternalInput")
    gw_d = nc.dram_tensor("gate_W", [D, E], F32, kind="ExternalInput")
    gb_d = nc.dram_tensor("gate_b", [E], F32, kind="ExternalInput")
    w1_d = nc.dram_tensor("W1", [E, D, D], F32, kind="ExternalInput")
    b1_d = nc.dram_tensor("b1", [E, D], F32, kind="ExternalInput")
    g1_d = nc.dram_tensor("g1", [E, D], F32, kind="ExternalInput")
    be1_d = nc.dram_tensor("be1", [E, D], F32, kind="ExternalInput")
    w2_d = nc.dram_tensor("W2", [E, D, D], F32, kind="ExternalInput")
    b2_d = nc.dram_tensor("b2", [E, D], F32, kind="ExternalInput")
    g2_d = nc.dram_tensor("g2", [E, D], F32, kind="ExternalInput")
    be2_d = nc.dram_tensor("be2", [E, D], F32, kind="ExternalInput")
    out_d = nc.dram_tensor("out", [T, D], F32, kind="ExternalOutput")

    with tile.TileContext(nc) as tc:
        with (
            tc.tile_pool(name="const", bufs=1) as const,
            tc.tile_pool(name="w1p", bufs=12) as w1p,
            tc.tile_pool(name="w2p", bufs=12) as w2p,
            tc.tile_pool(name="repp", bufs=2) as repp,
            tc.tile_pool(name="bvep", bufs=2) as bvep,
            tc.tile_pool(name="accp", bufs=TT) as accp,
            tc.tile_pool(name="workp", bufs=2) as workp,
            tc.tile_pool(name="xinp", bufs=2) as xinp,
            tc.tile_pool(name="statp", bufs=3) as statp,
            tc.tile_pool(name="gstp", bufs=1) as gstp,
        ):
            # ---- constants ----
            id_f32 = const.tile([P, P], F32)
            make_identity(nc, id_f32)
            id_bf16 = const.tile([P, P], BF16)
            make_identity(nc, id_bf16)
            ones_bf = const.tile([1, P], BF16)
            nc.vector.memset(ones_bf, 1.0)
            ones_f32 = const.tile([1, P], F32)
            nc.vector.memset(ones_f32, 1.0)
            eps_sb = const.tile([P, 1], F32)
            nc.vector.memset(eps_sb, LN_EPS)

            gw_sb = const.tile([P, KC, E], F32)
            nc.sync.dma_start(out=gw_sb, in_=gw_d.rearrange("(c p) e -> p c e", p=P))
            gb_sb = const.tile([1, E], F32)
            nc.sync.dma_start(out=gb_sb, in_=_row1(gb_d[:]))

            be2_sb = const.tile([E, D], BF16)
            nc.gpsimd.dma_start(out=be2_sb, in_=be2_d[:, :])  # casting dma

            xt_sb = const.tile([P, KC, T], BF16)  # x^T, matmul lhsT layout
            scores_sb = const.tile([P, TT, E], F32)
            w_sb = const.tile([P, TT, E], F32)
            wT_sb = const.tile([E, TT, P], BF16)

            w1tiles = {}
            w2tiles = {}
            bves = {}

            def load_w_chunk(e, c):
                t1w = w1p.tile([P, D], BF16, tag="w1", name=f"w1_{e}_{c}")
                nc.gpsimd.dma_start(out=t1w, in_=w1_d[e, ts(c, P), :])
                w1tiles[(e, c)] = t1w
                t2w = w2p.tile([P, D], BF16, tag="w2", name=f"w2_{e}_{c}")
                nc.gpsimd.dma_start(out=t2w, in_=w2_d[e, ts(c, P), :])
                w2tiles[(e, c)] = t2w

            for _c in range(KC):
                load_w_chunk(0, _c)

            # ---- prologue: transpose x, gate scores ----
            pre_ctx = tc.tile_pool(name="prep", bufs=2, space="PSUM")
            prep = pre_ctx.__enter__()
            for tt in range(TT):
                xin = xinp.tile([P, D], F32, tag="xin")
                nc.sync.dma_start(out=xin, in_=x_d[ts(tt, P), :])
                tp = prep.tile([P, D], F32, tag="tp")
                for c in range(KC):
                    nc.tensor.transpose(tp[:, ts(c, P)], xin[:, ts(c, P)], id_f32)
                xtg = workp.tile([P, D], F32, tag="n1")
                nc.scalar.copy(out=xtg, in_=tp)
                nc.vector.tensor_copy(
                    out=xt_sb[:, :, ts(tt, P)],
                    in_=tp.rearrange("p (c q) -> p c q", c=KC),
                )
                gps = prep.tile([P, E], F32, tag="gate")
                for c in range(KC):
                    nc.tensor.matmul(
                        gps,
                        xtg[:, ts(c, P)],
                        gw_sb[:, c, :],
                        start=(c == 0),
                        stop=False,
                    )
                nc.tensor.matmul(gps, ones_f32, gb_sb, start=False, stop=True)
                nc.scalar.copy(out=scores_sb[:, tt, :], in_=gps)

            # ---- top-2 softmax over the E=4 scores ----
            s3 = scores_sb  # [P, TT, E]
            m1 = gstp.tile([P, TT], F32, tag="m1")
            nc.vector.tensor_reduce(out=m1, in_=s3, axis=AX.X, op=ALU.max)
            m1b = m1.broadcast_to((P, TT, E))
            eqt = gstp.tile([P, TT, E], F32, tag="eqt")
            nc.vector.tensor_tensor(out=eqt, in0=s3, in1=m1b, op=ALU.is_equal)
            smt = gstp.tile([P, TT, E], F32, tag="smt")
            nc.vector.scalar_tensor_tensor(
                out=smt, in0=eqt, scalar=-1e30, in1=s3, op0=ALU.mult, op1=ALU.add
            )
            m2 = gstp.tile([P, TT], F32, tag="m2")
            nc.vector.tensor_reduce(out=m2, in_=smt, axis=AX.X, op=ALU.max)
            m2b = m2.broadcast_to((P, TT, E))
            ind = gstp.tile([P, TT, E], F32, tag="ind")
            nc.vector.tensor_tensor(out=ind, in0=s3, in1=m2b, op=ALU.is_ge)
            dd = gstp.tile([P, TT, E], F32, tag="dd")
            nc.vector.tensor_tensor(out=dd, in0=s3, in1=m1b, op=ALU.subtract)
            ex = gstp.tile([P, TT, E], F32, tag="ex")
            nc.scalar.activation(out=ex, in_=dd, func=AF.Exp)
            en = gstp.tile([P, TT, E], F32, tag="en")
            nc.vector.tensor_tensor(out=en, in0=ex, in1=ind, op=ALU.mult)
            zs = gstp.tile([P, TT], F32, tag="zs")
            nc.vector.tensor_reduce(out=zs, in_=en, axis=AX.X, op=ALU.add)
            rz = gstp.tile([P, TT], F32, tag="rz")
            nc.vector.reciprocal(out=rz, in_=zs)
            rzb = rz.broadcast_to((P, TT, E))
            nc.vector.tensor_tensor(out=w_sb, in0=en, in1=rzb, op=ALU.mult)
            for tt in range(TT):
                wtp = prep.tile([E, P], F32, tag="gate")
                nc.tensor.transpose(wtp, w_sb[:, tt, :], id_f32)
                nc.scalar.copy(out=wT_sb[:, tt, :], in_=wtp)

            pre_ctx.__exit__(None, None, None)
            zp_ctx = tc.tile_pool(name="zp", bufs=2, space="PSUM")
            zp = zp_ctx.__enter__()
            z2p_ctx = tc.tile_pool(name="z2p", bufs=1, space="PSUM")
            z2p = z2p_ctx.__enter__()
            utp_ctx = tc.tile_pool(name="utp", bufs=2, space="PSUM")
            utp = utp_ctx.__enter__()

            # ---- dense expert loop ----
            acc = {}

            def load_bve(e):
                bve = bvep.tile([1, 2, D], BF16, tag="bve", name=f"bve_{e}")
                nc.gpsimd.dma_start(out=bve[:, 0, :], in_=_row1(b1_d[e, :]))
                nc.gpsimd.dma_start(out=bve[:, 1, :], in_=_row1(b2_d[e, :]))
                bves[e] = bve

            reps = {}

            def load_reps(e):
                g1r = repp.tile([P, D], BF16, tag="g1r", name=f"g1r_{e}")
                nc.gpsimd.dma_start(out=g1r, in_=_bcast_rows(g1_d[e : e + 1, :]))
                be1r = repp.tile([P, D], BF16, tag="be1r", name=f"be1r_{e}")
                nc.gpsimd.dma_start(out=be1r, in_=_bcast_rows(be1_d[e : e + 1, :]))
                g2r = repp.tile([P, D], BF16, tag="g2r", name=f"g2r_{e}")
                nc.gpsimd.dma_start(out=g2r, in_=_bcast_rows(g2_d[e : e + 1, :]))
                reps[e] = (g1r, be1r, g2r)

            PREFETCH = 4  # chunks of expert e+1 issued inside expert e's loop
            for e in range(E):
                if e not in reps:
                    load_reps(e)
                g1r, be1r, g2r = reps[e]
                if e not in bves:
                    load_bve(e)
                for c in range(KC):
                    if (e, c) not in w1tiles:
                        load_w_chunk(e, c)
                w1t = [w1tiles[(e, c)] for c in range(KC)]
                w2t = [w2tiles[(e, c)] for c in range(KC)]
                bve = bves[e]

                for tt in range(TT):
                    if e + 1 < E and TT - PREFETCH - 1 <= tt < TT - 1:
                        pc = tt - (TT - PREFETCH - 1)
                        if (e + 1, pc) not in w1tiles:
                            load_w_chunk(e + 1, pc)
                    if e + 1 < E and tt == TT - 2 and (e + 1) not in reps:
                        load_reps(e + 1)
                    if e + 1 < E and tt == TT - 1 and (e + 1) not in bves:
                        load_bve(e + 1)
                    # --- z = x @ W1 + b1 ---
                    z = zp.tile([P, D], F32, tag="z")
                    for c in range(KC):
                        for n in range(NCH):
                            nc.tensor.matmul(
                                z[:, ds(n * 512, 512)],
                                xt_sb[:, c, ts(tt, P)],
                                w1t[c][:, ds(n * 512, 512)],
                                start=(c == 0),
                                stop=False,
                            )
                    for n in range(NCH):
                        nc.tensor.matmul(
                            z[:, ds(n * 512, 512)],
                            ones_bf,
                            bve[:, 0, ds(n * 512, 512)],
                            start=False,
                            stop=True,
                        )
                    # --- LN1 stats ---
                    st1 = statp.tile([P, 2, 6], F32, tag="st1")
                    nc.vector.bn_stats(out=st1[:, 0, :], in_=z[:, 0:512])
                    nc.vector.bn_stats(out=st1[:, 1, :], in_=z[:, 512:1024])
                    mv1 = statp.tile([P, 2], F32, tag="mv1")
                    nc.vector.bn_aggr(out=mv1, in_=st1)
                    sd1 = statp.tile([P, 1], F32, tag="sd1")
                    nc.scalar.activation(
                        out=sd1, in_=mv1[:, 1:2], func=AF.Sqrt, bias=eps_sb
                    )
                    rs1 = statp.tile([P, 1], F32, tag="rs1")
                    nc.vector.reciprocal(out=rs1, in_=sd1)
                    nmr1 = statp.tile([P, 1], F32, tag="nmr1")
                    nc.vector.tensor_scalar(
                        out=nmr1,
                        in0=mv1[:, 0:1],
                        scalar1=rs1,
                        scalar2=-1.0,
                        op0=ALU.mult,
                        op1=ALU.mult,
                    )
                    # --- u = relu((z - m)*rstd*g1 + be1) ---
                    n1 = workp.tile([P, D], F32, tag="n1")
                    nc.scalar.activation(
                        out=n1, in_=z, func=AF.Identity, bias=nmr1, scale=rs1
                    )
                    nc.vector.tensor_tensor(out=n1, in0=n1, in1=g1r, op=ALU.mult)
                    nc.gpsimd.tensor_tensor(out=n1, in0=n1, in1=be1r, op=ALU.add)
                    u = workp.tile([P, D], BF16, tag="u")
                    nc.scalar.activation(out=u, in_=n1, func=AF.Relu)
                    # --- u^T via PE ---
                    utps = utp.tile([P, D], BF16, tag="utp_bf")
                    for c in range(KC):
                        nc.tensor.transpose(utps[:, ts(c, P)], u[:, ts(c, P)], id_bf16)
                    uT = workp.tile([P, KC, P], BF16, tag="uT")
                    utv = utps.rearrange("p (c q) -> p c q", c=KC)
                    nc.scalar.copy(out=uT[:, 0 : KC // 2, :], in_=utv[:, 0 : KC // 2, :])
                    nc.vector.tensor_copy(
                        out=uT[:, KC // 2 :, :], in_=utv[:, KC // 2 :, :]
                    )
                    # --- z2 = u @ W2 + b2 ---
                    z2 = z2p.tile([P, D], F32, tag="z2")
                    for c in range(KC):
                        for n in range(NCH):
                            nc.tensor.matmul(
                                z2[:, ds(n * 512, 512)],
                                uT[:, c, :],
                                w2t[c][:, ds(n * 512, 512)],
                                start=(c == 0),
                                stop=False,
                            )
                    for n in range(NCH):
                        nc.tensor.matmul(
                            z2[:, ds(n * 512, 512)],
                            ones_bf,
                            bve[:, 1, ds(n * 512, 512)],
                            start=False,
                            stop=True,
                        )
                    # --- LN2 stats ---
                    st2 = statp.tile([P, 2, 6], F32, tag="st2")
                    nc.vector.bn_stats(out=st2[:, 0, :], in_=z2[:, 0:512])
                    nc.vector.bn_stats(out=st2[:, 1, :], in_=z2[:, 512:1024])
                    mv2 = statp.tile([P, 2], F32, tag="mv2")
                    nc.vector.bn_aggr(out=mv2, in_=st2)
                    sd2 = statp.tile([P, 1], F32, tag="sd2")
                    nc.scalar.activation(
                        out=sd2, in_=mv2[:, 1:2], func=AF.Sqrt, bias=eps_sb
                    )
                    rs2 = statp.tile([P, 1], F32, tag="rs2")
                    nc.vector.reciprocal(out=rs2, in_=sd2)
                    rw = statp.tile([P, 1], F32, tag="rw")
                    nc.vector.tensor_scalar_mul(
                        out=rw, in0=rs2, scalar1=w_sb[:, tt, e : e + 1]
                    )
                    nmr2 = statp.tile([P, 1], F32, tag="nmr2")
                    nc.vector.tensor_scalar(
                        out=nmr2,
                        in0=mv2[:, 0:1],
                        scalar1=rw,
                        scalar2=-1.0,
                        op0=ALU.mult,
                        op1=ALU.mult,
                    )
                    # --- y_e = (z2 - m2)*rstd2*w_e*g2 ; acc += y_e ---
                    n2 = workp.tile([P, D], F32, tag="n2")
                    nc.scalar.activation(
                        out=n2, in_=z2, func=AF.Identity, bias=nmr2, scale=rw
                    )
                    nc.vector.tensor_tensor(out=n2, in0=n2, in1=g2r, op=ALU.mult)
                    if e == 0:
                        xres = xinp.tile([P, D], F32, tag="xin")
                        nc.sync.dma_start(out=xres, in_=x_d[ts(tt, P), :])
                        acc[tt] = accp.tile([P, D], F32, tag="acc", name=f"acc_{tt}")
                        nc.gpsimd.tensor_tensor(
                            out=acc[tt], in0=n2, in1=xres, op=ALU.add
                        )
                    else:
                        nc.gpsimd.tensor_tensor(
                            out=acc[tt], in0=n2, in1=acc[tt], op=ALU.add
                        )
            utp_ctx.__exit__(None, None, None)
            z2p_ctx.__exit__(None, None, None)
            zp_ctx.__exit__(None, None, None)
            cpp_ctx = tc.tile_pool(name="cpp", bufs=2, space="PSUM")
            cpp = cpp_ctx.__enter__()

            # ---- finalize phase: out = acc + w @ be2 ----
            for tt in range(TT):
                outt = workp.tile([P, D], F32, tag="n1")
                for n in range(NCH):
                    cps = cpp.tile([P, 512], F32, tag="cp", name=f"cp_{tt}_{n}")
                    nc.tensor.matmul(
                        cps,
                        wT_sb[:, tt, :],
                        be2_sb[:, ds(n * 512, 512)],
                        start=True,
                        stop=True,
                    )
                    nc.vector.tensor_tensor(
                        out=outt[:, ds(n * 512, 512)],
                        in0=cps,
                        in1=acc[tt][:, ds(n * 512, 512)],
                        op=ALU.add,
                    )
                nc.sync.dma_start(out=out_d[ts(tt, P), :], in_=outt)

            cpp_ctx.__exit__(None, None, None)

    nc.compile()
    return nc


_nc_cache = {}
_nc_lock = threading.Lock()


def _get_nc(T, num_devices):
    key = (T, num_devices)
    with _nc_lock:
        if key not in _nc_cache:
            _nc_cache[key] = build_moe_nc(T, num_devices)
        return _nc_cache[key]


def kernel(**inputs) -> np.ndarray:
    from concourse.bass_utils import run_bass_kernel_spmd

    x = np.ascontiguousarray(np.asarray(inputs["x"], dtype=np.float32))
    B, N, Dd = x.shape
    assert Dd == D and B == N_CORES, (B, N, Dd)
    weights = {
        k: np.ascontiguousarray(np.asarray(inputs[k], dtype=np.float32))
        for k in (
            "gate_W",
            "gate_b",
            "W1",
            "b1",
            "g1",
            "be1",
            "W2",
            "b2",
            "g2",
            "be2",
        )
    }
    nc = _get_nc(N, N_CORES)
    in_maps = [dict(weights, x=x[i]) for i in range(N_CORES)]
    res = run_bass_kernel_spmd(nc, in_maps, core_ids=list(range(N_CORES)))
    out = np.stack([r["out"] for r in res.results], axis=0)
    return out.astype(np.float32)



# revision 62
# speedup vs baseline: 1.1256x; 1.1256x over previous
"""MoE (E=4 experts, top-2 routing) forward pass on 8 Trainium2 NeuronCores.

Strategy: data-parallel over tokens (core i processes batch row i = 2048
tokens), with TRUE top-2 routing on-device: each token's expert pair comes
from the fp32 gate scores; tokens are compacted per expert with tile-major
slot ids built from three small PE matmuls (strict-upper-triangular
partition prefix + column-sum + broadcast) and scattered via one masked
multi-row indirect DMA per expert into per-expert DRAM batches; each
expert runs its 2-layer FFN over its ~C=1152-token batch (vs 2048 dense ->
~1.8x less matmul work); per-expert outputs are gathered back by slot and
accumulated with the softmax weights. Tile-major slot order makes slots
monotonic in token-tile index, so each expert's combine overlaps its OWN
compute using a host-validated pacing table (gather for token tile tt
issues once the first pace[e][tt] routed tiles are done).

Per routed tile (36 = 4 experts x 9 capacity tiles):
  xrtT = xbar DMA-transpose load of 128 gathered tokens (bf16, no PE)
  z    = x @ W1   PE bf16, fp32 PSUM (stationary = xrtT, moving = W1)
  u    = relu(LN1(z))   DVE bn_stats + ACT fused scale/bias+relu
  uT   PE transpose + ACT copy
  z2   = u @ W2   PE bf16
  y    = LN2(z2) -> bf16 -> DMA to yrt[C*e + slot]

Identity affine params (g=1, b=0 -- how this problem's inputs are built)
are folded away at kernel() call time by inspecting the numpy values; a
general fallback path applies them when nonzero. Routing/gating stays fp32
throughout (min top2/top3 score gap on this data ~2e-5). Capacity C and
the pacing table are input statistics computed on host; the device does
all routing. A different input recompiles with its own (C, pace).
"""

import threading
from contextlib import ExitStack

import numpy as np

import concourse.bass as bass
import concourse.mybir as mybir
import concourse.tile as tile
from concourse import bacc
from concourse.bass import ds, ts
from concourse.masks import make_identity, make_upper_triangular

F32 = mybir.dt.float32
BF16 = mybir.dt.bfloat16
I32 = mybir.dt.int32
AF = mybir.ActivationFunctionType
ALU = mybir.AluOpType
AX = mybir.AxisListType

P = 128
D = 1024
E = 4
KC = D // P  # contraction chunks per matmul
NCH = D // 512  # psum column chunks
LN_EPS = 1e-5
N_CORES = 8
OOB = 1 << 20  # slot offset masking unselected (expert, token) pairs


def _row1(ap):
    """Lift an AP to have a leading length-1 (partition) dim."""
    return bass.AP(tensor=ap.tensor, offset=ap.offset, ap=[[0, 1]] + list(ap.ap))


def _bcast_rows(ap_row, p=P):
    """Broadcast a [1, N]-ish DRAM AP across p partitions (step-0 partition dim)."""
    inner = [list(d) for d in ap_row.ap if d[1] != 1]
    return bass.AP(tensor=ap_row.tensor, offset=ap_row.offset, ap=[[0, p]] + inner)


def build_moe_nc(T=2048, C=1152, pace=None, flags=(), num_devices=N_CORES):
    """pace[e][k]: number of expert-e routed tiles that must be complete
    before token-tile pair (2k, 2k+1) may gather expert e's output
    (host-validated upper bound). flags: the non-identity affine params."""
    TT = T // P
    R = C // P  # routed tiles per expert
    NIT = E * R  # total routed tiles
    has = set(flags)
    if pace is None:
        pace = tuple(tuple(R for _ in range(TT // 2)) for _ in range(E))
    nc = bacc.Bacc(
        "TRN2", target_bir_lowering=False, debug=False, num_devices=num_devices
    )

    x_d = nc.dram_tensor("x", [T, D], F32, kind="ExternalInput")
    gw_d = nc.dram_tensor("gate_W", [D, E], F32, kind="ExternalInput")
    gb_d = nc.dram_tensor("gate_b", [E], F32, kind="ExternalInput")
    w1_d = nc.dram_tensor("W1", [E, D, D], F32, kind="ExternalInput")
    b1_d = nc.dram_tensor("b1", [E, D], F32, kind="ExternalInput")
    g1_d = nc.dram_tensor("g1", [E, D], F32, kind="ExternalInput")
    be1_d = nc.dram_tensor("be1", [E, D], F32, kind="ExternalInput")
    w2_d = nc.dram_tensor("W2", [E, D, D], F32, kind="ExternalInput")
    b2_d = nc.dram_tensor("b2", [E, D], F32, kind="ExternalInput")
    g2_d = nc.dram_tensor("g2", [E, D], F32, kind="ExternalInput")
    be2_d = nc.dram_tensor("be2", [E, D], F32, kind="ExternalInput")
    out_d = nc.dram_tensor("out", [T, D], F32, kind="ExternalOutput")

    with tile.TileContext(nc) as tc:
        with ExitStack() as stack:
            ep = stack.enter_context
            const = ep(tc.tile_pool(name="const", bufs=1))
            drampx = ep(tc.tile_pool(name="dramx", bufs=1, space="DRAM"))
            drampy = ep(tc.tile_pool(name="dramy", bufs=1, space="DRAM"))
            xfp = ep(tc.tile_pool(name="xfp", bufs=3))
            xbfp = ep(tc.tile_pool(name="xbfp", bufs=1))
            xtgp = ep(tc.tile_pool(name="xtgp", bufs=2))
            routep = ep(tc.tile_pool(name="routep", bufs=1))
            gstp = ep(tc.tile_pool(name="gstp", bufs=1))
            top2p = ep(tc.tile_pool(name="top2p", bufs=2))
            wp = ep(tc.tile_pool(name="wp", bufs=2))
            repp = ep(tc.tile_pool(name="repp", bufs=1 if flags else 2))
            bvep = ep(tc.tile_pool(name="bvep", bufs=2))
            xrtTp = ep(tc.tile_pool(name="xrtTp", bufs=3))
            workp = ep(tc.tile_pool(name="workp", bufs=2))
            statp = ep(tc.tile_pool(name="statp", bufs=3))
            accp = ep(tc.tile_pool(name="accp", bufs=TT))
            gp = ep(tc.tile_pool(name="gp", bufs=3))
            combp = ep(tc.tile_pool(name="combp", bufs=2))

            # ---- constants ----
            id_f32 = const.tile([P, P], F32)
            make_identity(nc, id_f32)
            id_bf16 = const.tile([P, P], BF16)
            make_identity(nc, id_bf16)
            utri = const.tile([P, P], F32)
            make_upper_triangular(nc, utri, val=1.0, diag=False)  # U[q,p]=1 iff q<p
            ones_bf = const.tile([1, P], BF16)
            nc.vector.memset(ones_bf, 1.0)
            ones_f32 = const.tile([1, P], F32)
            nc.vector.memset(ones_f32, 1.0)
            ones_col = const.tile([P, 1], F32)
            nc.vector.memset(ones_col, 1.0)
            eps_sb = const.tile([P, 1], F32)
            nc.vector.memset(eps_sb, LN_EPS)
            ce_pe = const.tile([P, E], F32)  # [0, C, 2C, 3C] per partition
            for e in range(E):
                nc.vector.memset(ce_pe[:, e : e + 1], float(C * e))

            gw_sb = const.tile([P, KC, E], F32)
            nc.sync.dma_start(out=gw_sb, in_=gw_d.rearrange("(c p) e -> p c e", p=P))
            gb_sb = const.tile([1, E], F32)
            nc.sync.dma_start(out=gb_sb, in_=_row1(gb_d[:]))

            # routed token batches (bf16): per-expert tensors so expert e's
            # loads wait only on expert e's scatter
            xrt = [
                drampx.tile([C, D], BF16, tag=f"xrt{e}", name=f"xrt{e}")
                for e in range(E)
            ]
            yrt = drampy.tile([E * C, D], BF16, tag="yrt", name="yrt")

            # ---- expert weight loads (bf16 casting DMA on gpsimd) ----
            w1sb = {}
            w2sb = {}

            def load_w(e, which):
                # 4 chunks so no single transfer holds the DMA engines long
                # enough to starve the latency-critical xrtT loads
                src = w1_d if which == 1 else w2_d
                t = wp.tile([P, KC, D], BF16, tag=f"w{which}", name=f"w{which}_{e}")
                h = KC // 4
                for q in range(4):
                    nc.gpsimd.dma_start(
                        out=t[:, q * h : (q + 1) * h, :],
                        in_=src[e, ds(q * h * P, h * P), :].rearrange(
                            "(c p) n -> p c n", p=P
                        ),
                    )
                (w1sb if which == 1 else w2sb)[e] = t

            bves = {}
            reps = {}

            def load_bve(e):
                if not (has & {"b1", "b2"}):
                    return
                bve = bvep.tile([1, 2, D], BF16, tag="bve", name=f"bve_{e}")
                nc.gpsimd.dma_start(out=bve[:, 0, :], in_=_row1(b1_d[e, :]))
                nc.gpsimd.dma_start(out=bve[:, 1, :], in_=_row1(b2_d[e, :]))
                bves[e] = bve

            def load_reps(e):
                if not (has & {"g1", "be1", "g2", "be2"}):
                    return
                tiles = {}
                for nm, src in (("g1", g1_d), ("be1", be1_d), ("g2", g2_d), ("be2", be2_d)):
                    if nm in has:
                        t = repp.tile([P, D], BF16, tag=nm, name=f"{nm}_{e}")
                        nc.gpsimd.dma_start(out=t, in_=_bcast_rows(src[e : e + 1, :]))
                        tiles[nm] = t
                reps[e] = tiles

            # ---- prologue: stream x, fp32 gating (PE software-pipelined) ----
            pre_ctx = tc.tile_pool(name="prep", bufs=2, space="PSUM")
            prep = pre_ctx.__enter__()
            po_ctx = tc.tile_pool(name="pop", bufs=1, space="PSUM")
            pop = po_ctx.__enter__()

            xbf = xbfp.tile([P, TT, D], BF16, tag="xbf", name="xbf")
            scores_sb = const.tile([P, TT, E], F32)
            indT = routep.tile([P, E, TT], F32, tag="indT")
            nonselT = routep.tile([P, E, TT], F32, tag="nonselT")
            w_sb = gstp.tile([P, TT, E], F32, tag="w_sb")
            xtgs = {}

            def top2_chunk(a, b):
                # top-2 softmax over E for token tiles [a, b): emitted as the
                # scores arrive so the whole thing hides under the x stream
                n = b - a
                sl = scores_sb[:, a:b, :]
                m1c = top2p.tile([P, n], F32, tag="m1")
                nc.vector.tensor_reduce(out=m1c, in_=sl, axis=AX.X, op=ALU.max)
                m1bc = m1c.broadcast_to((P, n, E))
                ind1c = top2p.tile([P, n, E], F32, tag="ind1")
                nc.vector.tensor_tensor(out=ind1c, in0=sl, in1=m1bc, op=ALU.is_equal)
                smtc = top2p.tile([P, n, E], F32, tag="smt")
                nc.vector.scalar_tensor_tensor(
                    out=smtc, in0=ind1c, scalar=-1e30, in1=sl,
                    op0=ALU.mult, op1=ALU.add,
                )
                m2c = top2p.tile([P, n], F32, tag="m2")
                nc.vector.tensor_reduce(out=m2c, in_=smtc, axis=AX.X, op=ALU.max)
                m2bc = m2c.broadcast_to((P, n, E))
                indv = indT[:, :, a:b].rearrange("p e t -> p t e")
                nc.vector.tensor_tensor(out=indv, in0=sl, in1=m2bc, op=ALU.is_ge)
                nc.vector.tensor_tensor(
                    out=nonselT[:, :, a:b].rearrange("p e t -> p t e"),
                    in0=sl, in1=m2bc, op=ALU.is_lt,
                )
                ddc = top2p.tile([P, n, E], F32, tag="dd")
                nc.vector.tensor_tensor(out=ddc, in0=sl, in1=m1bc, op=ALU.subtract)
                exc = top2p.tile([P, n, E], F32, tag="ex")
                nc.scalar.activation(out=exc, in_=ddc, func=AF.Exp)
                enc = top2p.tile([P, n, E], F32, tag="en")
                nc.vector.tensor_tensor(out=enc, in0=exc, in1=indv, op=ALU.mult)
                zsc = top2p.tile([P, n], F32, tag="zs")
                nc.vector.tensor_reduce(out=zsc, in_=enc, axis=AX.X, op=ALU.add)
                rzc = top2p.tile([P, n], F32, tag="rz")
                nc.vector.reciprocal(out=rzc, in_=zsc)
                nc.vector.tensor_tensor(
                    out=w_sb[:, a:b, :], in0=enc,
                    in1=rzc.broadcast_to((P, n, E)), op=ALU.mult,
                )

            def gate_mm(tt):
                gps = prep.tile([P, E], F32, tag="gate")
                for c in range(KC):
                    nc.tensor.matmul(
                        gps,
                        xtgs[tt][:, ts(c, P)],
                        gw_sb[:, c, :],
                        start=(c == 0),
                        stop=(c == KC - 1 and "gb" not in has),
                    )
                if "gb" in has:
                    nc.tensor.matmul(gps, ones_f32, gb_sb, start=False, stop=True)
                nc.vector.tensor_copy(out=scores_sb[:, tt, :], in_=gps)
                del xtgs[tt]

            for tt in range(TT):
                xf = xfp.tile([P, D], F32, tag="xf")
                nc.sync.dma_start(out=xf, in_=x_d[ts(tt, P), :])
                nc.scalar.copy(out=xbf[:, tt, :], in_=xf)
                tp = prep.tile([P, D], F32, tag="tp")
                for c in range(KC):
                    nc.tensor.transpose(tp[:, ts(c, P)], xf[:, ts(c, P)], id_f32)
                xtg = xtgp.tile([P, D], F32, tag="xtg")
                nc.vector.tensor_copy(out=xtg, in_=tp)
                xtgs[tt] = xtg
                # gate matmuls run one tile behind the transposes so the PE
                # never waits on the DVE psum->sbuf copy
                if tt >= 1:
                    gate_mm(tt - 1)
                    if tt % 4 == 0:
                        top2_chunk(tt - 4, tt)
                if tt == TT - 1:
                    gate_mm(tt)
                    top2_chunk(TT - 4, TT)
                    # W transfers staged off the gating-critical x stream
                    with tc.tile_wait_until(0.024):
                        load_w(0, 1)
                    with tc.tile_wait_until(0.030):
                        load_w(0, 2)
                        load_bve(0)
                        load_reps(0)
                    with tc.tile_wait_until(0.042):
                        load_w(1, 1)
                        load_w(1, 2)
                        load_bve(1)
                        load_reps(1)

            # ---- routing: tile-major slot ids via 3 small PE matmuls ----
            # slot[p,tt,e] = sum_{t'<tt} colcnt[t',e] + sum_{q<p} ind[q,tt,e]
            iview = indT[:, :, :].rearrange("p e t -> p (e t)")  # [128, E*TT]
            cs_ps = pop.tile([1, E * TT], F32, tag="cs")
            nc.tensor.matmul(cs_ps, ones_col, iview, start=True, stop=True)
            cs = routep.tile([1, E, TT], F32, tag="cs_sb")
            nc.vector.tensor_copy(out=cs.rearrange("o e t -> o (e t)"), in_=cs_ps)
            # inclusive prefix over tt (single-partition shifted adds)
            cum = [
                routep.tile([1, E, TT], F32, tag=f"cum{i}", name=f"cum{i}")
                for i in range(2)
            ]
            ca = cs
            k = 1
            i = 0
            while k < TT:
                cb = cum[i % 2]
                nc.vector.tensor_tensor(
                    out=cb[:, :, k:], in0=ca[:, :, k:], in1=ca[:, :, : TT - k], op=ALU.add
                )
                nc.vector.tensor_copy(out=cb[:, :, :k], in_=ca[:, :, :k])
                ca = cb
                k *= 2
                i += 1
            tbase = routep.tile([1, E, TT], F32, tag="tbase")  # exclusive
            nc.vector.tensor_tensor(out=tbase, in0=ca, in1=cs, op=ALU.subtract)
            # slot = (partition prefix) + (tile base broadcast to 128 rows),
            # accumulated in one PSUM group
            slotp = pop.tile([P, E * TT], F32, tag="slotp")
            nc.tensor.matmul(slotp, utri, iview, start=True, stop=False)
            nc.tensor.matmul(
                slotp,
                ones_f32,
                tbase.rearrange("o e t -> o (e t)"),
                start=False,
                stop=True,
            )
            slot_loc = routep.tile([P, E, TT], F32, tag="slot_loc")
            nc.vector.tensor_copy(
                out=slot_loc.rearrange("p e t -> p (e t)"), in_=slotp
            )
            # scatter offsets: local slot, OOB-masked for unselected
            scfT = routep.tile([P, E, TT], F32, tag="scfT")
            nc.vector.scalar_tensor_tensor(
                out=scfT, in0=nonselT, scalar=float(OOB), in1=slot_loc,
                op0=ALU.mult, op1=ALU.add,
            )
            giSC = routep.tile([P, E * TT], I32, tag="giSC")
            nc.vector.tensor_copy(
                out=giSC.rearrange("p (e t) -> p e t", e=E), in_=scfT
            )
            giSCv = giSC.rearrange("p (e t) -> p e t", e=E)
            # gather offsets: global slot (+C*e), same mask
            gfT = routep.tile([P, E, TT], F32, tag="gfT")
            nc.vector.tensor_tensor(
                out=gfT, in0=scfT, in1=ce_pe.broadcast_to((P, E, TT)), op=ALU.add
            )
            giE = routep.tile([P, E * TT], I32, tag="giE")
            nc.vector.tensor_copy(
                out=giE.rearrange("p (e t) -> p e t", e=E), in_=gfT
            )
            giEv = giE.rearrange("p (e t) -> p e t", e=E)

            # ---- scatters: one masked indirect DMA per (expert, token tile)
            # (multi-offset indirect DMAs give wrong results on real HW, so
            # offsets stay [P, 1]). Expert 0's 16 run first so its compute
            # starts while experts 1-3 scatter. Out window [0:P] bounds the
            # per-tile rows so the cost model matches the real transfer.
            for e in range(E):
                for tt in range(TT):
                    nc.gpsimd.indirect_dma_start(
                        out=xrt[e][0:P, :],
                        out_offset=bass.IndirectOffsetOnAxis(
                            ap=giSCv[:, e, tt : tt + 1], axis=0
                        ),
                        in_=xbf[:, tt, :],
                        in_offset=None,
                        bounds_check=C - 1,
                        oob_is_err=False,
                    )

            po_ctx.__exit__(None, None, None)
            pre_ctx.__exit__(None, None, None)

            # ---- routed expert compute, software-pipelined by one tile ----
            zp_ctx = tc.tile_pool(name="zp", bufs=2, space="PSUM")
            zp = zp_ctx.__enter__()
            z2p_ctx = tc.tile_pool(name="z2p", bufs=1, space="PSUM")
            z2p = z2p_ctx.__enter__()
            utp_ctx = tc.tile_pool(name="utp", bufs=2, space="PSUM")
            utp = utp_ctx.__enter__()

            xrtT = {}
            zt = {}
            ut = {}
            uts = {}
            z2t = {}

            def s_load(j):
                e, r = divmod(j, R)
                t = xrtTp.tile([P, KC, P], BF16, tag="xrtT", name=f"xrtT_{j}")
                nc.sync.dma_start_transpose(out=t, in_=xrt[e][ds(r * P, P), :])
                xrtT[j] = t

            def s_z(j):
                e, r = divmod(j, R)
                z = zp.tile([P, D], F32, tag="z", name=f"z_{j}")
                last = KC - 1
                for c in range(KC):
                    for n in range(NCH):
                        nc.tensor.matmul(
                            z[:, ds(n * 512, 512)],
                            xrtT[j][:, c, :],
                            w1sb[e][:, c, ds(n * 512, 512)],
                            start=(c == 0),
                            stop=(c == last and "b1" not in has),
                        )
                if "b1" in has:
                    for n in range(NCH):
                        nc.tensor.matmul(
                            z[:, ds(n * 512, 512)],
                            ones_bf,
                            bves[e][:, 0, ds(n * 512, 512)],
                            start=False,
                            stop=True,
                        )
                zt[j] = z
                del xrtT[j]

            def s_ln1(j):
                e, r = divmod(j, R)
                z = zt[j]
                st1 = statp.tile([P, 2, 6], F32, tag="st1")
                nc.vector.bn_stats(out=st1[:, 0, :], in_=z[:, 0:512])
                nc.vector.bn_stats(out=st1[:, 1, :], in_=z[:, 512:1024])
                mv1 = statp.tile([P, 2], F32, tag="mv1")
                nc.vector.bn_aggr(out=mv1, in_=st1)
                sd1 = statp.tile([P, 1], F32, tag="sd1")
                nc.scalar.activation(
                    out=sd1, in_=mv1[:, 1:2], func=AF.Sqrt, bias=eps_sb
                )
                rs1 = statp.tile([P, 1], F32, tag="rs1")
                nc.vector.reciprocal(out=rs1, in_=sd1)
                nmr1 = statp.tile([P, 1], F32, tag="nmr1")
                nc.vector.tensor_scalar(
                    out=nmr1,
                    in0=mv1[:, 0:1],
                    scalar1=rs1,
                    scalar2=-1.0,
                    op0=ALU.mult,
                    op1=ALU.mult,
                )
                simple = not (has & {"g1", "be1"})
                u = workp.tile([P, D], BF16, tag="u", name=f"u_{j}")
                if simple:
                    nc.scalar.activation(
                        out=u, in_=z, func=AF.Relu, bias=nmr1, scale=rs1
                    )
                else:
                    n1 = workp.tile([P, D], F32, tag="ng")
                    nc.scalar.activation(
                        out=n1, in_=z, func=AF.Identity, bias=nmr1, scale=rs1
                    )
                    if "g1" in has:
                        nc.vector.tensor_tensor(
                            out=n1, in0=n1, in1=reps[e]["g1"], op=ALU.mult
                        )
                    if "be1" in has:
                        nc.gpsimd.tensor_tensor(
                            out=n1, in0=n1, in1=reps[e]["be1"], op=ALU.add
                        )
                    nc.scalar.activation(out=u, in_=n1, func=AF.Relu)
                ut[j] = u
                del zt[j]

            def s_tu(j):
                u = ut[j]
                utps = utp.tile([P, D], BF16, tag="uT", name=f"uT_{j}")
                for c in range(KC):
                    nc.tensor.transpose(utps[:, ts(c, P)], u[:, ts(c, P)], id_bf16)
                t = workp.tile([P, KC, P], BF16, tag="uTs", name=f"uTs_{j}")
                nc.scalar.copy(out=t, in_=utps.rearrange("p (c q) -> p c q", c=KC))
                uts[j] = t
                del ut[j]

            def s_z2(j):
                e, r = divmod(j, R)
                z2 = z2p.tile([P, D], F32, tag="z2", name=f"z2_{j}")
                last = KC - 1
                for c in range(KC):
                    for n in range(NCH):
                        nc.tensor.matmul(
                            z2[:, ds(n * 512, 512)],
                            uts[j][:, c, :],
                            w2sb[e][:, c, ds(n * 512, 512)],
                            start=(c == 0),
                            stop=(c == last and "b2" not in has),
                        )
                if "b2" in has:
                    for n in range(NCH):
                        nc.tensor.matmul(
                            z2[:, ds(n * 512, 512)],
                            ones_bf,
                            bves[e][:, 1, ds(n * 512, 512)],
                            start=False,
                            stop=True,
                        )
                z2t[j] = z2
                del uts[j]

            def s_ln2(j):
                e, r = divmod(j, R)
                z2 = z2t[j]
                st2 = statp.tile([P, 2, 6], F32, tag="st2")
                nc.vector.bn_stats(out=st2[:, 0, :], in_=z2[:, 0:512])
                nc.vector.bn_stats(out=st2[:, 1, :], in_=z2[:, 512:1024])
                mv2 = statp.tile([P, 2], F32, tag="mv2")
                nc.vector.bn_aggr(out=mv2, in_=st2)
                sd2 = statp.tile([P, 1], F32, tag="sd2")
                nc.scalar.activation(
                    out=sd2, in_=mv2[:, 1:2], func=AF.Sqrt, bias=eps_sb
                )
                rs2 = statp.tile([P, 1], F32, tag="rs2")
                nc.vector.reciprocal(out=rs2, in_=sd2)
                nmr2 = statp.tile([P, 1], F32, tag="nmr2")
                nc.vector.tensor_scalar(
                    out=nmr2,
                    in0=mv2[:, 0:1],
                    scalar1=rs2,
                    scalar2=-1.0,
                    op0=ALU.mult,
                    op1=ALU.mult,
                )
                simple = not (has & {"g2", "be2"})
                y = workp.tile([P, D], BF16, tag="y", name=f"y_{j}")
                if simple:
                    nc.scalar.activation(
                        out=y, in_=z2, func=AF.Identity, bias=nmr2, scale=rs2
                    )
                else:
                    n2 = workp.tile([P, D], F32, tag="ng")
                    nc.scalar.activation(
                        out=n2, in_=z2, func=AF.Identity, bias=nmr2, scale=rs2
                    )
                    if "g2" in has:
                        nc.vector.tensor_tensor(
                            out=n2, in0=n2, in1=reps[e]["g2"], op=ALU.mult
                        )
                    if "be2" in has:
                        nc.gpsimd.tensor_tensor(
                            out=n2, in0=n2, in1=reps[e]["be2"], op=ALU.add
                        )
                    nc.scalar.copy(out=y, in_=n2)
                del z2t[j]
                # on the ACT hwdge queue: ACT produced y just above, and this
                # keeps the wait off SP where it would block xrtT loads
                nc.scalar.dma_start(out=yrt[ds(C * e + r * P, P), :], in_=y)

            # combine: per-expert accumulation paced by the host-computed
            # slot bounds, overlapped with the same expert's compute
            acc = {}
            n_comb = [0]

            def s_gather1(e, tt):
                ge = gp.tile([P, D], BF16, tag="g", name=f"g_{e}_{tt}")
                if n_comb[0] < 3:
                    # first pool rotations read uninitialized SBUF: OOB-skipped
                    # gather rows would otherwise hold junk (NaN risk) that
                    # w=0 cannot neutralize
                    nc.vector.memset(ge, 0.0)
                n_comb[0] += 1
                nc.gpsimd.indirect_dma_start(
                    out=ge,
                    out_offset=None,
                    in_=yrt[:, :],
                    in_offset=bass.IndirectOffsetOnAxis(
                        ap=giEv[:, e, tt : tt + 1], axis=0
                    ),
                    bounds_check=E * C - 1,
                    oob_is_err=False,
                )
                return ge

            def s_comb1(e, tt, ge):
                t = combp.tile([P, D], BF16, tag="t", name=f"t_{e}_{tt}")
                nc.scalar.activation(
                    out=t, in_=ge, func=AF.Identity,
                    scale=w_sb[:, tt, e : e + 1],
                )
                if e == 0:
                    a = accp.tile([P, D], BF16, tag="acc", name=f"acc_{tt}")
                    nc.vector.tensor_tensor(
                        out=a, in0=t, in1=xbf[:, tt, :], op=ALU.add
                    )
                    acc[tt] = a
                elif e < E - 1:
                    nc.vector.tensor_tensor(
                        out=acc[tt], in0=acc[tt], in1=t, op=ALU.add
                    )
                else:
                    o = combp.tile([P, D], F32, tag="o", name=f"o_{tt}")
                    nc.vector.tensor_tensor(out=o, in0=acc[tt], in1=t, op=ALU.add)
                    nc.sync.dma_start(out=out_d[ts(tt, P), :], in_=o)

            # combine schedule: (e, tt) runs at loop index e*R + pace
            comb_at = {}
            for e in range(E):
                for k in range(TT // 2):
                    jj = e * R + min(max(int(pace[e][k]), 1), R)
                    comb_at.setdefault(jj, []).append((e, 2 * k))
                    comb_at.setdefault(jj, []).append((e, 2 * k + 1))

            s_load(0)
            s_load(1)
            for j in range(NIT + 1):
                if j + 2 < NIT:
                    s_load(j + 2)
                if j >= 1:
                    s_tu(j - 1)
                if j < NIT:
                    s_z(j)
                if j >= 1:
                    s_z2(j - 1)
                if j < NIT:
                    s_ln1(j)
                if j >= 1:
                    s_ln2(j - 1)
                # gathers before combines: an out-write emitted earlier would
                # falsely serialize later (DRAM-aliasing) indirect gathers
                pend = comb_at.get(j, [])
                ges = [s_gather1(e, tt) for e, tt in pend]
                for (e, tt), ge in zip(pend, ges):
                    s_comb1(e, tt, ge)
                if j < NIT:
                    e, r = divmod(j, R)
                    if r == min(5, R - 1) and 2 <= e + 1 < E:
                        load_w(e + 1, 1)
                        load_bve(e + 1)
                        load_reps(e + 1)
                    if r == min(6, R - 1) and 2 <= e + 1 < E:
                        load_w(e + 1, 2)

            # leftover combines (pace == R for the last expert's last tiles)
            for jj in sorted(k for k in comb_at if k > NIT):
                pend = comb_at[jj]
                ges = [s_gather1(e, tt) for e, tt in pend]
                for (e, tt), ge in zip(pend, ges):
                    s_comb1(e, tt, ge)

            utp_ctx.__exit__(None, None, None)
            z2p_ctx.__exit__(None, None, None)
            zp_ctx.__exit__(None, None, None)

    nc.compile()
    return nc


_nc_cache = {}
_nc_lock = threading.Lock()


def _get_nc(T, C, pace, flags, num_devices):
    key = (T, C, pace, flags, num_devices)
    with _nc_lock:
        if key not in _nc_cache:
            _nc_cache[key] = build_moe_nc(T, C, pace, flags, num_devices)
        return _nc_cache[key]


def _route_stats(x, gate_W, gate_b):
    """Capacity C and combine pacing (host-side shape/schedule decisions
    only -- all routing happens on-device). Tile-major slot order must
    match the device computation."""
    B, N, _ = x.shape
    TT = N // P
    mx = 0
    pace = np.zeros((E, TT), int)
    for b in range(B):
        gs = x[b].astype(np.float32) @ gate_W + gate_b
        top2 = np.argpartition(-gs, 2, axis=-1)[:, :2]
        sel = np.zeros((N, E), bool)
        for k in range(2):
            sel[np.arange(N), top2[:, k]] = True
        selt = sel.reshape(TT, P, E)
        csum = selt.sum(axis=1).cumsum(axis=0)
        mx = max(mx, int(csum[-1].max()))
        need = (csum + P - 1) // P  # routed tiles needed after token tile tt
        pace = np.maximum(pace, need.T)
    C = ((mx + 64 + P - 1) // P) * P
    pace = np.minimum(pace, C // P)
    pace_pairs = tuple(
        tuple(int(max(pace[e, 2 * k], pace[e, 2 * k + 1])) for k in range(TT // 2))
        for e in range(E)
    )
    return C, pace_pairs


def kernel(**inputs) -> np.ndarray:
    from concourse.bass_utils import run_bass_kernel_spmd

    x = np.ascontiguousarray(np.asarray(inputs["x"], dtype=np.float32))
    B, N, Dd = x.shape
    assert Dd == D and B == N_CORES, (B, N, Dd)
    weights = {
        k: np.ascontiguousarray(np.asarray(inputs[k], dtype=np.float32))
        for k in (
            "gate_W",
            "gate_b",
            "W1",
            "b1",
            "g1",
            "be1",
            "W2",
            "b2",
            "g2",
            "be2",
        )
    }
    flags = []
    if np.any(weights["gate_b"] != 0):
        flags.append("gb")
    for nm in ("b1", "b2", "be1", "be2"):
        if np.any(weights[nm] != 0):
            flags.append(nm)
    for nm in ("g1", "g2"):
        if np.any(weights[nm] != 1):
            flags.append(nm)
    C, pace = _route_stats(x, weights["gate_W"], weights["gate_b"])
    nc = _get_nc(N, C, pace, tuple(sorted(flags)), N_CORES)
    in_maps = [dict(weights, x=x[i]) for i in range(N_CORES)]
    res = run_bass_kernel_spmd(nc, in_maps, core_ids=list(range(N_CORES)))
    out = np.stack([r["out"] for r in res.results], axis=0)
    return out.astype(np.float32)
